# revision 1
# baseline (speedup 1.0000x reference)
"""Trainium2 Bass kernel for nn_CustomActivation (knot-GELU).

y = 0.5*x*(1 + tanh(sqrt(2/pi) * (x + 0.044715 * (m*(m+1))**3))),  m = ceil(x)

Strategy (memory-bound target):
  - Shard x (8, 8192, 2048) f32 along axis 0 across the 8 NeuronCores;
    pure data parallel, no communication.
  - Per core, the whole computation is 3 VectorE instructions + 1 ScalarE
    instruction per tile:
      1. r  = (x + 1.5*2^23) - 1.5*2^23        stock tensor_scalar (2x mode)
              -> round-to-nearest-even of x (exact for |x| < 2^22)
      2. z  = x + (cbrt(c) * m * (m+1))**3     custom 8-stage DVE op, where
              m = r + (x > r)  == exact ceil(x)
      3. th = Tanh(sqrt(2/pi) * z)             ScalarE activation
      4. y  = (th + 1) * x * 0.5               custom 3-stage DVE op
  - Per-core engine busy (theoretical): DVE ~340us, ACT ~110us vs
    HBM roofline ~373us -> memory bound.
"""

import math
import sys

sys.path.insert(0, "/opt/trn_rl_repo")

import numpy as np

N_CORES = 8
B, T, D = 8, 8192, 2048          # full input shape
P = 128                          # SBUF partitions
F = 4096                         # free-dim elements per tile
NT = (T * D) // (P * F)          # tiles per core (32)

MAGIC = 12582912.0               # 1.5 * 2^23: RNE-to-integer magic constant
GELU_COEF = 0.044715
CBRT_C = float(GELU_COEF ** (1.0 / 3.0))
SQRT_2_OVER_PI = math.sqrt(2.0 / math.pi)

_state = {}


def _register_ops():
    """Define + register the two custom DVE ops (idempotent)."""
    import concourse.dve_ops as dve_ops_mod
    from concourse.dve_ops import DveOp
    from concourse.dve_spec import Spec, Src0, Src1, C0, One, lower, _has_src1
    from concourse.dve_uop import DveOpSpec

    if "KNOT_Z_ANT" in dve_ops_mod._SUB_OPCODE_FOR_NAME:
        by_name = {op.name: op for op in dve_ops_mod.OPS}
        return by_name["KNOT_Z_ANT"], by_name["KNOT_COMBINE_ANT"]

    def _knot_z_ref(in0, in1, s0, s1, imm2):
        x = in0.astype(np.float32)
        r = in1.astype(np.float32)
        g = (x > r).astype(np.float32)
        m = r + g
        a = m * np.float32(s0)
        b = m + np.float32(1.0)
        p = a * b
        return ((p * p) * p + x).astype(np.float32)

    # z = x + (C0*m * (m+1))^3 with m = r + (x > r); in0 = x, in1 = r
    g = Src0 > Src1
    m = Src1 + g
    a = m * C0
    b = m + One
    p = a * b
    q = p * p
    w = q * p
    spec_z = Spec(body=w + Src0, reference=_knot_z_ref)

    def _combine_ref(in0, in1, s0, s1, imm2):
        th = in0.astype(np.float32)
        x = in1.astype(np.float32)
        return ((th + np.float32(1.0)) * x * np.float32(s0)).astype(np.float32)

    # y = (th + 1) * x * C0; in0 = th, in1 = x
    spec_c = Spec(body=(Src0 + One) * Src1 * C0, reference=_combine_ref)

    ops = []
    for name, spec in (("KNOT_Z_ANT", spec_z), ("KNOT_COMBINE_ANT", spec_c)):
        shas = {}
        for ver in ("v3", "v4"):
            tmp = DveOpSpec(name=name, uops=lower(spec, ver=ver),
                            rd1_en=_has_src1(spec))
            shas[ver] = tmp.sha(ver)
        op = DveOp(name, spec, subdim=False, uops_sha=shas)
        dve_ops_mod.OPS.append(op)
        dve_ops_mod._SUB_OPCODE_FOR_NAME[name] = (
            dve_ops_mod._CUSTOM_DVE_ROW_BASE + len(dve_ops_mod.OPS) - 1
        )
        assert dve_ops_mod._SUB_OPCODE_FOR_NAME[name] < 0x20
        dve_ops_mod.CUSTOM_DVE_SPECS[name] = spec
        ops.append(op)
    return ops[0], ops[1]


def _build():
    """Build + compile the per-core Bass program (cached)."""
    if "nc" in _state:
        return _state["nc"]

    import concourse.bacc as bacc
    import concourse.mybir as mybir
    import concourse.tile as tile

    knot_z, knot_combine = _register_ops()

    f32 = mybir.dt.float32
    nc = bacc.Bacc("TRN2", target_bir_lowering=False, debug=False,
                   num_devices=N_CORES)
    x_d = nc.dram_tensor("x", [NT, P, F], f32, kind="ExternalInput").ap()
    y_d = nc.dram_tensor("out", [NT, P, F], f32, kind="ExternalOutput").ap()

    # Work list: (tile_idx, load_chunks, compute_chunks, rne_on_dve).
    # Tile 0 is loaded + computed in 1 MiB quarters so compute starts
    # ~1.5 us after the first bytes land; the last two tiles are loaded
    # whole (DMA-efficient) but computed + stored in quarters so the
    # drain exposes only a short chain + small store. The RNE rounding
    # runs on ScalarE (2 exact Copy activations) for most tiles and on
    # VectorE (fused tensor_scalar) for enough of them to balance the
    # two engines well under the DMA roofline.
    work = []
    dve_rne = {5, 10, 15, 20, 25}  # full tiles whose RNE runs on VectorE
    for i in range(NT):
        if i in (0, NT - 2, NT - 1):
            for c in range(4):
                work.append((i, c * (F // 4), F // 4, True))
        else:
            work.append((i, 0, F, i in dve_rne))

    def rne(out_ap, in_ap, on_dve):
        if on_dve:
            nc.vector.tensor_scalar(
                out=out_ap, in0=in_ap, scalar1=MAGIC, scalar2=MAGIC,
                op0=mybir.AluOpType.add, op1=mybir.AluOpType.subtract,
            )
        else:
            nc.scalar.activation(
                out=out_ap, in_=in_ap,
                func=mybir.ActivationFunctionType.Copy, bias=MAGIC, scale=1.0,
            )
            nc.scalar.activation(
                out=out_ap, in_=out_ap,
                func=mybir.ActivationFunctionType.Copy, bias=-MAGIC, scale=1.0,
            )

    with tile.TileContext(nc) as tc:
        with (
            tc.tile_pool(name="xp", bufs=6) as xp,
            tc.tile_pool(name="rp", bufs=6) as rp,
        ):
            for i, off, n, rne_dve in work:
                xt = xp.tile([P, n], f32, tag="x")
                nc.gpsimd.dma_start(out=xt[:], in_=x_d[i, :, off:off + n])

                # r -> z -> th -> y all in-place in one working tile: each
                # op reads the previous stage's value and streams over it.
                rt = rp.tile([P, n], f32, tag="r")
                rne(rt[:], xt[:], rne_dve)
                nc.vector._custom_dve(knot_z, out=rt[:], in0=xt[:],
                                      in1=rt[:], s0=CBRT_C)
                nc.scalar.activation(
                    out=rt[:], in_=rt[:],
                    func=mybir.ActivationFunctionType.Tanh,
                    scale=SQRT_2_OVER_PI,
                )
                nc.vector._custom_dve(knot_combine, out=rt[:],
                                      in0=rt[:], in1=xt[:], s0=0.5)
                nc.sync.dma_start(out=y_d[i, :, off:off + n], in_=rt[:])

    nc.compile()
    _state["nc"] = nc
    return nc


def run(x: np.ndarray, **spmd_kwargs):
    """Run the SPMD kernel on the full input; returns (y_full, results)."""
    from concourse.bass_utils import run_bass_kernel_spmd

    nc = _build()
    x = np.ascontiguousarray(np.asarray(x), dtype=np.float32)
    assert x.shape == (B, T, D), x.shape
    shards = x.reshape(N_CORES, NT, P, F)
    in_maps = [{"x": shards[i]} for i in range(N_CORES)]
    res = run_bass_kernel_spmd(nc, in_maps, core_ids=list(range(N_CORES)),
                               **spmd_kwargs)
    y = np.stack([res.results[i]["out"].reshape(T, D)
                  for i in range(N_CORES)])
    return y.astype(np.float32, copy=False), res


def kernel(x: np.ndarray) -> np.ndarray:
    y, _ = run(x)
    return y

